# revision 13
# baseline (speedup 1.0000x reference)
"""Bahdanau additive attention on 8 Trainium2 NeuronCores.

c[b] = softmax_t( tanh(s@W_a + h@U_a) @ v_a ) @ h[b]

Sharding: data-parallel over batch B=32 -> 4 batches per core; W_a, U_a,
v_a replicated. Everything per-core is a single pass over h[b] (64 MiB
f32) using an unnormalized softmax (scores are bounded by ||v_a||_1, so
exp() in f32 never overflows and no running max is needed).

Per-core pipeline, per (batch, t-chunk of 1024):
  1. SWDGE DMA loads h chunk HBM->SBUF casting f32->bf16 ([t_lo, th, dh]).
  2. XBAR DMA-transpose makes the dh-major copy hT [dh_lo,(th,dh_hi),t_lo].
  3. PE: scores_pre[a, t] += U_a[dh,a].T @ hT (8 dh-tiles accum in PSUM).
  4. ACT: tanh(psum + bias(W_a@s)) -> SBUF bf16 (per 128-a tile).
  5. XBAR-transpose tanh -> t-major [t_lo, th, a].
  6. DVE: e[t_lo, th] = sum_a tanhT * v_a  (fused tensor_tensor_reduce).
  7. ACT: p = exp(e) -> bf16, accum_out gives the softmax denominator.
  8. PE: c[1, dh] += p[t_lo,th].T @ h_bf16  (PSUM accum across chunks).
  9. Finalize per batch: l = sum(p) via ones-matmul, c * (1/l) -> out.
"""

import numpy as np

B, T, DH, DS, DA = 32, 4096, 1024, 1024, 512
NCORES = 8
BL = B // NCORES          # batches per core
CHUNK_T = 1024            # timesteps per pipeline chunk
NCHUNK = T // CHUNK_T
TH = CHUNK_T // 128       # 128-row subtiles per chunk
P = 128

_CACHE = {}


def _legalize_waits(nc):
    """This walrus build allows at most one sync wait per instruction.
    Tile's tail drain (and any instruction whose operands arrive via two
    DMA lanes) can carry several; split the extras onto single-wait nops
    emitted just before, in the same engine's stream."""
    from concourse import mybir

    eng_map = {}
    for eng_name in ("sync", "tensor", "vector", "scalar", "gpsimd"):
        eng = getattr(nc, eng_name)
        eng_map[eng.engine] = eng

    def make_nop(engine_type):
        bi = eng_map[engine_type].nop(nofuse=True)
        inst = bi.ins
        # pop it from whatever block it was appended to
        for fn in nc.m.functions:
            for blk in fn.blocks:
                il = list(blk.instructions)
                if il and il[-1].name == inst.name:
                    blk.instructions = il[:-1]
                    return inst
        raise RuntimeError("nop not found after emit")

    for fn in nc.m.functions:
        for blk in fn.blocks:
            insts = list(blk.instructions)
            if not any(
                getattr(i, "sync_info", None) is not None
                and len(i.sync_info.on_wait) > 1
                for i in insts
            ):
                continue
            out = []
            for inst in insts:
                si = getattr(inst, "sync_info", None)
                if si is not None and len(si.on_wait) > 1:
                    waits = list(si.on_wait)
                    for w in waits[:-1]:
                        nop = make_nop(inst.engine)
                        nop.sync_info = mybir.SyncInfo(
                            on_wait=[w], on_update=[]
                        )
                        out.append(nop)
                    inst.sync_info = mybir.SyncInfo(
                        on_wait=[waits[-1]], on_update=list(si.on_update)
                    )
                out.append(inst)
            blk.instructions = out


def build_bass(bl=BL, t_total=T, stage=4):
    import concourse.bass as bass
    import concourse.tile as tile
    from concourse import mybir

    f32 = mybir.dt.float32
    bf16 = mybir.dt.bfloat16
    Alu = mybir.AluOpType
    Act = mybir.ActivationFunctionType
    Axis = mybir.AxisListType

    nchunk = t_total // CHUNK_T

    nc = bass.Bass()
    s_ext = nc.declare_dram_parameter("s", [bl, DS], f32, isOutput=False)
    h_ext = nc.declare_dram_parameter("h", [bl, t_total, DH], f32, isOutput=False)
    w_ext = nc.declare_dram_parameter("W_a", [DS, DA], f32, isOutput=False)
    u_ext = nc.declare_dram_parameter("U_a", [DH, DA], f32, isOutput=False)
    v_ext = nc.declare_dram_parameter("v_a", [DA], f32, isOutput=False)
    out_ext = nc.declare_dram_parameter("out", [bl, DH], f32, isOutput=True)

    with tile.TileContext(nc) as tc:
        from contextlib import ExitStack

        with ExitStack() as ctx:
            singles = ctx.enter_context(tc.tile_pool(name="singles", bufs=1))
            hpool = ctx.enter_context(tc.tile_pool(name="hpool", bufs=3))
            htpool = ctx.enter_context(tc.tile_pool(name="htpool", bufs=2))
            tanhpool = ctx.enter_context(tc.tile_pool(name="tanhpool", bufs=3))
            ttpool = ctx.enter_context(tc.tile_pool(name="ttpool", bufs=2))
            smpool = ctx.enter_context(tc.tile_pool(name="smpool", bufs=3))
            prodpool = ctx.enter_context(tc.tile_pool(name="prodpool", bufs=2))
            outpool = ctx.enter_context(tc.tile_pool(name="outpool", bufs=2))
            mm1ps = ctx.enter_context(
                tc.tile_pool(name="mm1ps", bufs=2, space="PSUM")
            )
            cps_pool = ctx.enter_context(
                tc.tile_pool(name="cps", bufs=1, space="PSUM")
            )
            tinyps = ctx.enter_context(
                tc.tile_pool(name="tinyps", bufs=1, space="PSUM")
            )

            # ---- one-time setup ----
            # U_a as bf16 [dh_lo, dh_hi, a] (lhsT tiles for the score matmul)
            # NB: SWDGE cast DMAs deadlock with 3D APs on this runtime —
            # keep every casting DMA 2D.
            u_sb = singles.tile([P, DH // P, DA], bf16)
            u_re = u_ext[:].rearrange("(o p) a -> p o a", p=P)
            for o in range(DH // P):
                nc.gpsimd.dma_start(u_sb[:, o, :], u_re[:, o, :])
            # W_a f32 [ds_lo, ds_hi, a] (lhsT tiles for the W_a@s matmul)
            w_sb = singles.tile([P, DS // P, DA], f32)
            nc.sync.dma_start(w_sb, w_ext[:].rearrange("(o p) a -> p o a", p=P))
            # s [bl, DS] f32
            s_sb = singles.tile([bl, DS], f32)
            nc.sync.dma_start(s_sb, s_ext[:])
            # v_a broadcast to all partitions (f32 via HWDGE), then cast
            v_f32 = singles.tile([P, DA // P, P], f32)
            nc.sync.dma_start(
                v_f32,
                v_ext[:].rearrange("(g a) -> g a", g=DA // P)[None].to_broadcast(
                    (P, DA // P, P)
                ),
            )
            v_bc = singles.tile([P, DA // P, P], bf16)
            nc.vector.tensor_copy(v_bc, v_f32)
            ones_sb = singles.tile([P, 1], f32)
            nc.any.memset(ones_sb, 1.0)

            # sT [ds_lo, ds_hi, b] via strided DMAs (16 KB, one-time)
            st_sb = singles.tile([P, DS // P, bl], f32)
            with nc.allow_non_contiguous_dma(
                reason="tiny one-time s transpose"
            ):
                for b in range(bl):
                    nc.gpsimd.dma_start(
                        st_sb[:, :, b],
                        s_ext[b].rearrange("(o p) -> p o", p=P),
                    )

            # W_a_s^T [a_lo, a_hi, b] = sum_ds W_a[ds, a] * s[b, ds]
            ps_ws = tinyps.tile([P, DA // P, bl], f32, tag="tiny")
            for at in range(DA // P):
                for o in range(DS // P):
                    nc.tensor.matmul(
                        ps_ws[:, at, :],
                        w_sb[:, o, at * P : (at + 1) * P],
                        st_sb[:, o, :],
                        start=(o == 0),
                        stop=(o == DS // P - 1),
                    )
            ws_sb = singles.tile([P, DA // P, bl], f32)
            nc.vector.tensor_copy(ws_sb, ps_ws)

            # ---- main loop ----
            def emit_load(b, i):
                # t within the chunk decomposes as t = tq*8 + tr, so the
                # cast DMA is 2D: partition tq strides 8 DRAM rows, and
                # (tr, d) is one contiguous 32 KB run.
                # hbf[tq, tr, d] = h[b, i*CHUNK_T + tq*8 + tr, d]
                hbf = hpool.tile([P, TH, DH], bf16, tag="hbf")
                nc.gpsimd.dma_start(
                    hbf.rearrange("p a b -> p (a b)"),
                    h_ext[
                        b, i * CHUNK_T : (i + 1) * CHUNK_T, :
                    ].rearrange("(tq tr) d -> tq (tr d)", tr=TH),
                )
                # xbar: ht[p, q, f] = hbf_2d[f, q*128+p]
                # => ht[dh_lo, (tr, o), tq] = h[.., tq*8 + tr, o*128 + dh_lo]
                ht = htpool.tile([P, TH, DH // P, P], bf16, tag="ht")
                nc.sync.dma_start_transpose(ht, hbf)
                return hbf, ht

            def emit_scores(b, i, ht, lparts):
                tt = ttpool.tile([P, TH, DA // P, P], bf16, tag="tt")
                for at in range(DA // P):
                    ps1 = mm1ps.tile([P, CHUNK_T], f32, tag="mm1")
                    for o in range(DH // P):
                        lhsT = u_sb[:, o, at * P : (at + 1) * P]
                        nc.tensor.matmul(
                            ps1[:, 0:512],
                            lhsT,
                            ht[:, 0 : TH // 2, o, :],
                            start=(o == 0),
                            stop=(o == DH // P - 1),
                        )
                        nc.tensor.matmul(
                            ps1[:, 512:1024],
                            lhsT,
                            ht[:, TH // 2 : TH, o, :],
                            start=(o == 0),
                            stop=(o == DH // P - 1),
                        )
                    tanh_sb = tanhpool.tile([P, CHUNK_T], bf16, tag="tanh")
                    nc.scalar.activation(
                        tanh_sb,
                        ps1,
                        Act.Tanh,
                        bias=ws_sb[:, at, b : b + 1],
                    )
                    if stage >= 3:
                        # ttT[q, th, at, p] = tanh[p, th*128 + q]
                        nc.sync.dma_start_transpose(tt[:, :, at, :], tanh_sb)
                if stage < 3:
                    return None

                et = smpool.tile([P, TH], f32, tag="et")
                for th in range(TH):
                    prod = prodpool.tile([P, DA // P, P], f32, tag="prod")
                    nc.vector.scalar_tensor_tensor(
                        out=prod,
                        in0=tt[:, th, :, :],
                        scalar=1.0,
                        in1=v_bc,
                        op0=Alu.mult,
                        op1=Alu.mult,
                        accum_out=et[:, th : th + 1],
                    )
                pt = smpool.tile([P, TH], bf16, tag="pt")
                nc.scalar.activation(
                    pt, et, Act.Exp, accum_out=lparts[:, i : i + 1]
                )
                return pt

            def emit_mm3(b, i, hbf, pt, cps):
                for th in range(TH):
                    first = i == 0 and th == 0
                    last = i == nchunk - 1 and th == TH - 1
                    nc.tensor.matmul(
                        cps[:, 0:512],
                        pt[:, th : th + 1],
                        hbf[:, th, 0:512],
                        start=first,
                        stop=last,
                    )
                    nc.tensor.matmul(
                        cps[:, 512:1024],
                        pt[:, th : th + 1],
                        hbf[:, th, 512:1024],
                        start=first,
                        stop=last,
                    )

            def emit_finalize(b, lparts, cps):
                lsum = smpool.tile([P, 1], f32, tag="lsum")
                nc.vector.tensor_reduce(
                    out=lsum, in_=lparts, axis=Axis.X, op=Alu.add
                )
                lps = tinyps.tile([1, 1], f32, tag="tiny")
                nc.tensor.matmul(lps, lsum, ones_sb, start=True, stop=True)
                rl = smpool.tile([1, 1], f32, tag="rl")
                nc.vector.reciprocal(rl, lps)
                o_sb = outpool.tile([1, DH], f32, tag="osb")
                nc.vector.tensor_scalar_mul(o_sb, cps, rl)
                nc.sync.dma_start(out_ext[b : b + 1, :], o_sb)

            pending = None
            for b in range(bl):
                lparts = smpool.tile([P, nchunk], f32, tag="lparts")
                cps = cps_pool.tile([1, DH], f32, tag="c")
                for i in range(nchunk):
                    hbf, ht = emit_load(b, i)
                    if stage >= 2:
                        pt = emit_scores(b, i, ht, lparts)
                    if stage >= 4:
                        if pending is not None:
                            emit_mm3(*pending)
                            if pending[1] == nchunk - 1:
                                emit_finalize(pending[0], *pending_fin)
                        pending = (b, i, hbf, pt, cps)
                pending_fin = (lparts, cps)
            if stage >= 4:
                emit_mm3(*pending)
                emit_finalize(pending[0], *pending_fin)
            else:
                for b in range(bl):
                    o_sb = outpool.tile([1, DH], f32, tag="osb")
                    nc.any.memset(o_sb, 0.0)
                    nc.sync.dma_start(out_ext[b : b + 1, :], o_sb)

    _legalize_waits(nc)
    return nc


def _get_nc():
    if "nc" not in _CACHE:
        _CACHE["nc"] = build_bass()
    return _CACHE["nc"]


def kernel(s, h, W_a, U_a, v_a):
    from concourse.bass_utils import run_bass_kernel_spmd

    s = np.ascontiguousarray(np.asarray(s, dtype=np.float32))
    h = np.ascontiguousarray(np.asarray(h, dtype=np.float32))
    W_a = np.ascontiguousarray(np.asarray(W_a, dtype=np.float32))
    U_a = np.ascontiguousarray(np.asarray(U_a, dtype=np.float32))
    v_a = np.ascontiguousarray(np.asarray(v_a, dtype=np.float32))

    nc = _get_nc()
    in_maps = []
    for c in range(NCORES):
        sl = slice(c * BL, (c + 1) * BL)
        in_maps.append(
            {"s": s[sl], "h": h[sl], "W_a": W_a, "U_a": U_a, "v_a": v_a}
        )
    res = run_bass_kernel_spmd(nc, in_maps, core_ids=list(range(NCORES)))
    outs = [res.results[c]["out"] for c in range(NCORES)]
    return np.concatenate(outs, axis=0).astype(np.float32)


# revision 21
# speedup vs baseline: 1.2325x; 1.2325x over previous
"""Bahdanau additive attention on 8 Trainium2 NeuronCores.

c[b] = softmax_t( tanh(s@W_a + h@U_a) @ v_a ) @ h[b]

Sharding: data-parallel over batch B=32 -> 4 batches per core; W_a, U_a,
v_a replicated. Everything per-core is a single pass over h[b] (64 MiB
f32) using an unnormalized softmax (scores are bounded by ||v_a||_1, so
exp() in f32 never overflows and no running max is needed).

Per-core pipeline, per (batch, t-chunk of 1024):
  1. SWDGE DMA loads h chunk HBM->SBUF casting f32->bf16 ([t_lo, th, dh]).
  2. XBAR DMA-transpose makes the dh-major copy hT [dh_lo,(th,dh_hi),t_lo].
  3. PE: scores_pre[a, t] += U_a[dh,a].T @ hT (8 dh-tiles accum in PSUM).
  4. ACT: tanh(psum + bias(W_a@s)) -> SBUF bf16 (per 128-a tile).
  5. XBAR-transpose tanh -> t-major [t_lo, th, a].
  6. DVE: e[t_lo, th] = sum_a tanhT * v_a  (fused tensor_tensor_reduce).
  7. ACT: p = exp(e) -> bf16, accum_out gives the softmax denominator.
  8. PE: c[1, dh] += p[t_lo,th].T @ h_bf16  (PSUM accum across chunks).
  9. Finalize per batch: l = sum(p) via ones-matmul, c * (1/l) -> out.
"""

import numpy as np

B, T, DH, DS, DA = 32, 4096, 1024, 1024, 512
NCORES = 8
BL = B // NCORES          # batches per core
CHUNK_T = 1024            # timesteps per pipeline chunk
NCHUNK = T // CHUNK_T
TH = CHUNK_T // 128       # 128-row subtiles per chunk
P = 128

_CACHE = {}


def _legalize_waits(nc):
    """This walrus build allows at most one sync wait per instruction.
    Tile's tail drain (and any instruction whose operands arrive via two
    DMA lanes) can carry several; split the extras onto single-wait nops
    emitted just before, in the same engine's stream."""
    from concourse import mybir

    eng_map = {}
    for eng_name in ("sync", "tensor", "vector", "scalar", "gpsimd"):
        eng = getattr(nc, eng_name)
        eng_map[eng.engine] = eng

    def make_nop(engine_type):
        bi = eng_map[engine_type].nop(nofuse=True)
        inst = bi.ins
        # pop it from whatever block it was appended to
        for fn in nc.m.functions:
            for blk in fn.blocks:
                il = list(blk.instructions)
                if il and il[-1].name == inst.name:
                    blk.instructions = il[:-1]
                    return inst
        raise RuntimeError("nop not found after emit")

    for fn in nc.m.functions:
        for blk in fn.blocks:
            insts = list(blk.instructions)
            if not any(
                getattr(i, "sync_info", None) is not None
                and len(i.sync_info.on_wait) > 1
                for i in insts
            ):
                continue
            out = []
            for inst in insts:
                si = getattr(inst, "sync_info", None)
                if si is not None and len(si.on_wait) > 1:
                    waits = list(si.on_wait)
                    for w in waits[:-1]:
                        nop = make_nop(inst.engine)
                        nop.sync_info = mybir.SyncInfo(
                            on_wait=[w], on_update=[]
                        )
                        out.append(nop)
                    inst.sync_info = mybir.SyncInfo(
                        on_wait=[waits[-1]], on_update=list(si.on_update)
                    )
                out.append(inst)
            blk.instructions = out


def build_bass(bl=BL, t_total=T, stage=4):
    import concourse.bass as bass
    import concourse.tile as tile
    from concourse import mybir

    f32 = mybir.dt.float32
    bf16 = mybir.dt.bfloat16
    Alu = mybir.AluOpType
    Act = mybir.ActivationFunctionType
    Axis = mybir.AxisListType

    nchunk = t_total // CHUNK_T

    nc = bass.Bass()
    s_ext = nc.declare_dram_parameter("s", [bl, DS], f32, isOutput=False)
    h_ext = nc.declare_dram_parameter("h", [bl, t_total, DH], f32, isOutput=False)
    w_ext = nc.declare_dram_parameter("W_a", [DS, DA], f32, isOutput=False)
    u_ext = nc.declare_dram_parameter("U_a", [DH, DA], f32, isOutput=False)
    v_ext = nc.declare_dram_parameter("v_a", [DA], f32, isOutput=False)
    out_ext = nc.declare_dram_parameter("out", [bl, DH], f32, isOutput=True)

    with tile.TileContext(nc) as tc:
        from contextlib import ExitStack

        with ExitStack() as ctx:
            singles = ctx.enter_context(tc.tile_pool(name="singles", bufs=1))
            hpool = ctx.enter_context(tc.tile_pool(name="hpool", bufs=3))
            htpool = ctx.enter_context(tc.tile_pool(name="htpool", bufs=2))
            tanhpool = ctx.enter_context(tc.tile_pool(name="tanhpool", bufs=5))
            smpool = ctx.enter_context(tc.tile_pool(name="smpool", bufs=3))
            outpool = ctx.enter_context(tc.tile_pool(name="outpool", bufs=2))
            mm1ps = ctx.enter_context(
                tc.tile_pool(name="mm1ps", bufs=2, space="PSUM")
            )
            cps_pool = ctx.enter_context(
                tc.tile_pool(name="cps", bufs=1, space="PSUM")
            )
            eps_pool = ctx.enter_context(
                tc.tile_pool(name="epsp", bufs=2, space="PSUM")
            )
            tinyps = eps_pool

            # ---- one-time setup ----
            # U_a as bf16 [dh_lo, dh_hi, a] (lhsT tiles for the score matmul)
            # NB: SWDGE cast DMAs deadlock with 3D APs on this runtime —
            # keep every casting DMA 2D.
            u_sb = singles.tile([P, DH // P, DA], bf16)
            u_re = u_ext[:].rearrange("(o p) a -> p o a", p=P)
            for o in range(DH // P):
                nc.gpsimd.dma_start(u_sb[:, o, :], u_re[:, o, :])
            # W_a f32 [ds_lo, ds_hi, a] (lhsT tiles for the W_a@s matmul)
            w_sb = singles.tile([P, DS // P, DA], f32)
            nc.sync.dma_start(w_sb, w_ext[:].rearrange("(o p) a -> p o a", p=P))
            # s [bl, DS] f32
            s_sb = singles.tile([bl, DS], f32)
            nc.sync.dma_start(s_sb, s_ext[:])
            # v_a as [a_lo, a_hi] f32 -> bf16 (rhs columns for the e-dot)
            v_f32 = singles.tile([P, DA // P], f32)
            with nc.allow_non_contiguous_dma(
                reason="tiny one-time v load"
            ):
                nc.gpsimd.dma_start(
                    v_f32, v_ext[:].rearrange("(g a) -> a g", g=DA // P)
                )
            v_bf = singles.tile([P, DA // P], bf16)
            nc.vector.tensor_copy(v_bf, v_f32)
            ones_sb = singles.tile([P, 1], f32)
            nc.any.memset(ones_sb, 1.0)

            # sT [ds_lo, ds_hi, b] via strided DMAs (16 KB, one-time)
            st_sb = singles.tile([P, DS // P, bl], f32)
            with nc.allow_non_contiguous_dma(
                reason="tiny one-time s transpose"
            ):
                for b in range(bl):
                    nc.gpsimd.dma_start(
                        st_sb[:, :, b],
                        s_ext[b].rearrange("(o p) -> p o", p=P),
                    )

            # W_a_s^T [a_lo, a_hi, b] = sum_ds W_a[ds, a] * s[b, ds]
            ps_ws = tinyps.tile([P, DA // P, bl], f32, tag="eps")
            for at in range(DA // P):
                for o in range(DS // P):
                    nc.tensor.matmul(
                        ps_ws[:, at, :],
                        w_sb[:, o, at * P : (at + 1) * P],
                        st_sb[:, o, :],
                        start=(o == 0),
                        stop=(o == DS // P - 1),
                    )
            ws_sb = singles.tile([P, DA // P, bl], f32)
            nc.vector.tensor_copy(ws_sb, ps_ws)

            # ---- main loop ----
            def emit_load(b, i):
                # t within the chunk decomposes as t = tq*8 + tr, so the
                # cast DMA is 2D: partition tq strides 8 DRAM rows, and
                # (tr, d) is one contiguous 32 KB run.
                # hbf[tq, tr, d] = h[b, i*CHUNK_T + tq*8 + tr, d]
                hbf = hpool.tile([P, TH, DH], bf16, tag="hbf")
                nc.gpsimd.dma_start(
                    hbf.rearrange("p a b -> p (a b)"),
                    h_ext[
                        b, i * CHUNK_T : (i + 1) * CHUNK_T, :
                    ].rearrange("(tq tr) d -> tq (tr d)", tr=TH),
                )
                # xbar: ht[p, q, f] = hbf_2d[f, q*128+p]
                # => ht[dh_lo, (tr, o), tq] = h[.., tq*8 + tr, o*128 + dh_lo]
                # Alternate the issuing HWDGE queue so transpose issue
                # time is not serialized on one sequencer.
                ht = htpool.tile([P, TH, DH // P, P], bf16, tag="ht")
                eng = nc.sync if i % 2 == 0 else nc.scalar
                eng.dma_start_transpose(ht, hbf)
                return hbf, ht

            def emit_scores(b, i, ht, lparts):
                # e_ps[tq, tr] accumulates sum_a v[a] * tanh[a, c=tr*128+tq]
                # via matmuls with the tanh c-tile as lhsT and v as rhs.
                eps = eps_pool.tile([P, TH], f32, tag="eps")
                tanhs = []

                for at in range(DA // P):
                    ps1 = mm1ps.tile([P, CHUNK_T], f32, tag="mm1")
                    for o in range(DH // P):
                        lhsT = u_sb[:, o, at * P : (at + 1) * P]
                        nc.tensor.matmul(
                            ps1[:, 0:512],
                            lhsT,
                            ht[:, 0 : TH // 2, o, :],
                            start=(o == 0),
                            stop=(o == DH // P - 1),
                        )
                        nc.tensor.matmul(
                            ps1[:, 512:1024],
                            lhsT,
                            ht[:, TH // 2 : TH, o, :],
                            start=(o == 0),
                            stop=(o == DH // P - 1),
                        )
                    tanh_sb = tanhpool.tile([P, CHUNK_T], bf16, tag="tanh")
                    nc.scalar.activation(
                        tanh_sb,
                        ps1,
                        Act.Tanh,
                        bias=ws_sb[:, at, b : b + 1],
                    )
                    tanhs.append(tanh_sb)
                if stage < 3:
                    return None
                # group-per-column: each eps column's accumulation over
                # the 4 a-tiles opens and closes before the next column
                for ct in range(TH):
                    for at in range(DA // P):
                        nc.tensor.matmul(
                            eps[:, ct : ct + 1],
                            tanhs[at][:, ct * P : (ct + 1) * P],
                            v_bf[:, at : at + 1],
                            start=(at == 0),
                            stop=(at == DA // P - 1),
                        )

                pt = smpool.tile([P, TH], bf16, tag="pt")
                nc.scalar.activation(
                    pt, eps, Act.Exp, accum_out=lparts[:, i : i + 1]
                )
                return pt

            def emit_mm3(b, i, hbf, pt, cps):
                for th in range(TH):
                    first = i == 0 and th == 0
                    last = i == nchunk - 1 and th == TH - 1
                    nc.tensor.matmul(
                        cps[:, 0:512],
                        pt[:, th : th + 1],
                        hbf[:, th, 0:512],
                        start=first,
                        stop=last,
                    )
                    nc.tensor.matmul(
                        cps[:, 512:1024],
                        pt[:, th : th + 1],
                        hbf[:, th, 512:1024],
                        start=first,
                        stop=last,
                    )

            def emit_finalize(b, lparts, cps):
                lsum = smpool.tile([P, 1], f32, tag="lsum")
                nc.vector.tensor_reduce(
                    out=lsum, in_=lparts, axis=Axis.X, op=Alu.add
                )
                lps = tinyps.tile([1, 1], f32, tag="eps")
                nc.tensor.matmul(lps, lsum, ones_sb, start=True, stop=True)
                rl = smpool.tile([1, 1], f32, tag="rl")
                nc.vector.reciprocal(rl, lps)
                o_sb = outpool.tile([1, DH], f32, tag="osb")
                nc.vector.tensor_scalar_mul(o_sb, cps, rl)
                nc.sync.dma_start(out_ext[b : b + 1, :], o_sb)

            pending = None
            for b in range(bl):
                lparts = smpool.tile([P, nchunk], f32, tag="lparts")
                cps = cps_pool.tile([1, DH], f32, tag="c")
                for i in range(nchunk):
                    hbf, ht = emit_load(b, i)
                    if stage >= 2:
                        pt = emit_scores(b, i, ht, lparts)
                    if stage >= 4:
                        if pending is not None:
                            emit_mm3(*pending)
                            if pending[1] == nchunk - 1:
                                emit_finalize(pending[0], *pending_fin)
                        pending = (b, i, hbf, pt, cps)
                pending_fin = (lparts, cps)
            if stage >= 4:
                emit_mm3(*pending)
                emit_finalize(pending[0], *pending_fin)
            else:
                for b in range(bl):
                    o_sb = outpool.tile([1, DH], f32, tag="osb")
                    nc.any.memset(o_sb, 0.0)
                    nc.sync.dma_start(out_ext[b : b + 1, :], o_sb)

    _legalize_waits(nc)
    return nc


def _get_nc():
    if "nc" not in _CACHE:
        _CACHE["nc"] = build_bass()
    return _CACHE["nc"]


def kernel(s, h, W_a, U_a, v_a):
    from concourse.bass_utils import run_bass_kernel_spmd

    s = np.ascontiguousarray(np.asarray(s, dtype=np.float32))
    h = np.ascontiguousarray(np.asarray(h, dtype=np.float32))
    W_a = np.ascontiguousarray(np.asarray(W_a, dtype=np.float32))
    U_a = np.ascontiguousarray(np.asarray(U_a, dtype=np.float32))
    v_a = np.ascontiguousarray(np.asarray(v_a, dtype=np.float32))

    nc = _get_nc()
    in_maps = []
    for c in range(NCORES):
        sl = slice(c * BL, (c + 1) * BL)
        in_maps.append(
            {"s": s[sl], "h": h[sl], "W_a": W_a, "U_a": U_a, "v_a": v_a}
        )
    res = run_bass_kernel_spmd(nc, in_maps, core_ids=list(range(NCORES)))
    outs = [res.results[c]["out"] for c in range(NCORES)]
    return np.concatenate(outs, axis=0).astype(np.float32)


# revision 24
# speedup vs baseline: 1.2654x; 1.0267x over previous
"""Bahdanau additive attention on 8 Trainium2 NeuronCores.

c[b] = softmax_t( tanh(s@W_a + h@U_a) @ v_a ) @ h[b]

Sharding: data-parallel over batch B=32 -> 4 batches per core; W_a, U_a,
v_a replicated. Everything per-core is a single pass over h[b] (64 MiB
f32) using an unnormalized softmax (scores are bounded by ||v_a||_1, so
exp() in f32 never overflows and no running max is needed).

Per-core pipeline, per (batch, t-chunk of 1024):
  1. SWDGE DMA loads h chunk HBM->SBUF casting f32->bf16 ([t_lo, th, dh]).
  2. XBAR DMA-transpose makes the dh-major copy hT [dh_lo,(th,dh_hi),t_lo].
  3. PE: scores_pre[a, t] += U_a[dh,a].T @ hT (8 dh-tiles accum in PSUM).
  4. ACT: tanh(psum + bias(W_a@s)) -> SBUF bf16 (per 128-a tile).
  5. XBAR-transpose tanh -> t-major [t_lo, th, a].
  6. DVE: e[t_lo, th] = sum_a tanhT * v_a  (fused tensor_tensor_reduce).
  7. ACT: p = exp(e) -> bf16, accum_out gives the softmax denominator.
  8. PE: c[1, dh] += p[t_lo,th].T @ h_bf16  (PSUM accum across chunks).
  9. Finalize per batch: l = sum(p) via ones-matmul, c * (1/l) -> out.
"""

import numpy as np

B, T, DH, DS, DA = 32, 4096, 1024, 1024, 512
NCORES = 8
BL = B // NCORES          # batches per core
CHUNK_T = 1024            # timesteps per pipeline chunk
NCHUNK = T // CHUNK_T
TH = CHUNK_T // 128       # 128-row subtiles per chunk
P = 128

_CACHE = {}


def _legalize_waits(nc):
    """This walrus build allows at most one sync wait per instruction.
    Tile's tail drain (and any instruction whose operands arrive via two
    DMA lanes) can carry several; split the extras onto single-wait nops
    emitted just before, in the same engine's stream."""
    from concourse import mybir

    eng_map = {}
    for eng_name in ("sync", "tensor", "vector", "scalar", "gpsimd"):
        eng = getattr(nc, eng_name)
        eng_map[eng.engine] = eng

    def make_nop(engine_type):
        bi = eng_map[engine_type].nop(nofuse=True)
        inst = bi.ins
        # pop it from whatever block it was appended to
        for fn in nc.m.functions:
            for blk in fn.blocks:
                il = list(blk.instructions)
                if il and il[-1].name == inst.name:
                    blk.instructions = il[:-1]
                    return inst
        raise RuntimeError("nop not found after emit")

    for fn in nc.m.functions:
        for blk in fn.blocks:
            insts = list(blk.instructions)
            if not any(
                getattr(i, "sync_info", None) is not None
                and len(i.sync_info.on_wait) > 1
                for i in insts
            ):
                continue
            out = []
            for inst in insts:
                si = getattr(inst, "sync_info", None)
                if si is not None and len(si.on_wait) > 1:
                    waits = list(si.on_wait)
                    for w in waits[:-1]:
                        nop = make_nop(inst.engine)
                        nop.sync_info = mybir.SyncInfo(
                            on_wait=[w], on_update=[]
                        )
                        out.append(nop)
                    inst.sync_info = mybir.SyncInfo(
                        on_wait=[waits[-1]], on_update=list(si.on_update)
                    )
                out.append(inst)
            blk.instructions = out


def build_bass(bl=BL, t_total=T, stage=4):
    import concourse.bass as bass
    import concourse.tile as tile
    from concourse import mybir

    f32 = mybir.dt.float32
    bf16 = mybir.dt.bfloat16
    Alu = mybir.AluOpType
    Act = mybir.ActivationFunctionType
    Axis = mybir.AxisListType

    nchunk = t_total // CHUNK_T

    nc = bass.Bass()
    s_ext = nc.declare_dram_parameter("s", [bl, DS], f32, isOutput=False)
    h_ext = nc.declare_dram_parameter("h", [bl, t_total, DH], f32, isOutput=False)
    w_ext = nc.declare_dram_parameter("W_a", [DS, DA], f32, isOutput=False)
    u_ext = nc.declare_dram_parameter("U_a", [DH, DA], f32, isOutput=False)
    v_ext = nc.declare_dram_parameter("v_a", [DA], f32, isOutput=False)
    out_ext = nc.declare_dram_parameter("out", [bl, DH], f32, isOutput=True)

    with tile.TileContext(nc) as tc:
        from contextlib import ExitStack

        with ExitStack() as ctx:
            singles = ctx.enter_context(tc.tile_pool(name="singles", bufs=1))
            hpool = ctx.enter_context(tc.tile_pool(name="hpool", bufs=4))
            htpool = ctx.enter_context(tc.tile_pool(name="htpool", bufs=3))
            tanhpool = ctx.enter_context(tc.tile_pool(name="tanhpool", bufs=5))
            smpool = ctx.enter_context(tc.tile_pool(name="smpool", bufs=3))
            outpool = ctx.enter_context(tc.tile_pool(name="outpool", bufs=2))
            mm1ps = ctx.enter_context(
                tc.tile_pool(name="mm1ps", bufs=2, space="PSUM")
            )
            cps_pool = ctx.enter_context(
                tc.tile_pool(name="cps", bufs=1, space="PSUM")
            )
            eps_pool = ctx.enter_context(
                tc.tile_pool(name="epsp", bufs=2, space="PSUM")
            )
            tinyps = eps_pool

            def emit_load(b, i):
                # t within the chunk decomposes as t = tq*8 + tr, so the
                # cast DMA is 2D: partition tq strides 8 DRAM rows, and
                # (tr, d) is one contiguous 32 KB run.
                # hbf[tq, tr, d] = h[b, i*CHUNK_T + tq*8 + tr, d]
                hbf = hpool.tile([P, TH, DH], bf16, tag="hbf")
                nc.gpsimd.dma_start(
                    hbf.rearrange("p a b -> p (a b)"),
                    h_ext[
                        b, i * CHUNK_T : (i + 1) * CHUNK_T, :
                    ].rearrange("(tq tr) d -> tq (tr d)", tr=TH),
                )
                # xbar: ht[p, q, f] = hbf_2d[f, q*128+p]
                # => ht[dh_lo, (tr, o), tq] = h[.., tq*8 + tr, o*128 + dh_lo]
                ht = htpool.tile([P, TH, DH // P, P], bf16, tag="ht")
                nc.sync.dma_start_transpose(ht, hbf)
                return hbf, ht

            # Start the h pipeline before the setup loads so the gpsimd
            # DMA queue is not stuck behind them.
            preload = {}
            for pb, pi in ((0, 0), (0, 1)):
                if pi < nchunk:
                    preload[(pb, pi)] = emit_load(pb, pi)

            # ---- one-time setup ----
            # U_a as bf16 [dh_lo, dh_hi, a] (lhsT tiles for the score matmul)
            # NB: SWDGE cast DMAs deadlock with 3D APs on this runtime —
            # keep every casting DMA 2D.
            u_sb = singles.tile([P, DH // P, DA], bf16)
            u_re = u_ext[:].rearrange("(o p) a -> p o a", p=P)
            for o in range(DH // P):
                nc.gpsimd.dma_start(u_sb[:, o, :], u_re[:, o, :])
            # W_a f32 [ds_lo, ds_hi, a] (lhsT tiles for the W_a@s matmul)
            w_sb = singles.tile([P, DS // P, DA], f32)
            nc.sync.dma_start(w_sb, w_ext[:].rearrange("(o p) a -> p o a", p=P))
            # s [bl, DS] f32
            s_sb = singles.tile([bl, DS], f32)
            nc.sync.dma_start(s_sb, s_ext[:])
            # v_a as [a_lo, a_hi] f32 -> bf16 (rhs columns for the e-dot)
            v_f32 = singles.tile([P, DA // P], f32)
            with nc.allow_non_contiguous_dma(
                reason="tiny one-time v load"
            ):
                nc.gpsimd.dma_start(
                    v_f32, v_ext[:].rearrange("(g a) -> a g", g=DA // P)
                )
            v_bf = singles.tile([P, DA // P], bf16)
            nc.vector.tensor_copy(v_bf, v_f32)
            ones_sb = singles.tile([P, 1], f32)
            nc.any.memset(ones_sb, 1.0)

            # sT [ds_lo, ds_hi, b] via strided DMAs (16 KB, one-time)
            st_sb = singles.tile([P, DS // P, bl], f32)
            with nc.allow_non_contiguous_dma(
                reason="tiny one-time s transpose"
            ):
                for b in range(bl):
                    nc.gpsimd.dma_start(
                        st_sb[:, :, b],
                        s_ext[b].rearrange("(o p) -> p o", p=P),
                    )

            # W_a_s^T [a_lo, a_hi, b] = sum_ds W_a[ds, a] * s[b, ds]
            ps_ws = tinyps.tile([P, DA // P, bl], f32, tag="eps")
            for at in range(DA // P):
                for o in range(DS // P):
                    nc.tensor.matmul(
                        ps_ws[:, at, :],
                        w_sb[:, o, at * P : (at + 1) * P],
                        st_sb[:, o, :],
                        start=(o == 0),
                        stop=(o == DS // P - 1),
                    )
            ws_sb = singles.tile([P, DA // P, bl], f32)
            nc.vector.tensor_copy(ws_sb, ps_ws)

            # ---- main loop ----
            def emit_scores(b, i, ht, lparts):
                # e_ps[tq, tr] accumulates sum_a v[a] * tanh[a, c=tr*128+tq]
                # via matmuls with the tanh c-tile as lhsT and v as rhs.
                eps = eps_pool.tile([P, TH], f32, tag="eps")
                tanhs = []

                for at in range(DA // P):
                    ps1 = mm1ps.tile([P, CHUNK_T], f32, tag="mm1")
                    for o in range(DH // P):
                        lhsT = u_sb[:, o, at * P : (at + 1) * P]
                        nc.tensor.matmul(
                            ps1[:, 0:512],
                            lhsT,
                            ht[:, 0 : TH // 2, o, :],
                            start=(o == 0),
                            stop=(o == DH // P - 1),
                        )
                        nc.tensor.matmul(
                            ps1[:, 512:1024],
                            lhsT,
                            ht[:, TH // 2 : TH, o, :],
                            start=(o == 0),
                            stop=(o == DH // P - 1),
                        )
                    tanh_sb = tanhpool.tile([P, CHUNK_T], bf16, tag="tanh")
                    nc.scalar.activation(
                        tanh_sb,
                        ps1,
                        Act.Tanh,
                        bias=ws_sb[:, at, b : b + 1],
                    )
                    tanhs.append(tanh_sb)
                if stage < 3:
                    return None
                # group-per-column: each eps column's accumulation over
                # the 4 a-tiles opens and closes before the next column
                for ct in range(TH):
                    for at in range(DA // P):
                        nc.tensor.matmul(
                            eps[:, ct : ct + 1],
                            tanhs[at][:, ct * P : (ct + 1) * P],
                            v_bf[:, at : at + 1],
                            start=(at == 0),
                            stop=(at == DA // P - 1),
                        )

                pt = smpool.tile([P, TH], bf16, tag="pt")
                nc.scalar.activation(
                    pt, eps, Act.Exp, accum_out=lparts[:, i : i + 1]
                )
                return pt

            def emit_mm3(b, i, hbf, pt, cps):
                for th in range(TH):
                    first = i == 0 and th == 0
                    last = i == nchunk - 1 and th == TH - 1
                    nc.tensor.matmul(
                        cps[:, 0:512],
                        pt[:, th : th + 1],
                        hbf[:, th, 0:512],
                        start=first,
                        stop=last,
                    )
                    nc.tensor.matmul(
                        cps[:, 512:1024],
                        pt[:, th : th + 1],
                        hbf[:, th, 512:1024],
                        start=first,
                        stop=last,
                    )

            def emit_finalize(b, lparts, cps):
                lsum = smpool.tile([P, 1], f32, tag="lsum")
                nc.vector.tensor_reduce(
                    out=lsum, in_=lparts, axis=Axis.X, op=Alu.add
                )
                lps = tinyps.tile([1, 1], f32, tag="eps")
                nc.tensor.matmul(lps, lsum, ones_sb, start=True, stop=True)
                rl = smpool.tile([1, 1], f32, tag="rl")
                nc.vector.reciprocal(rl, lps)
                o_sb = outpool.tile([1, DH], f32, tag="osb")
                nc.vector.tensor_scalar_mul(o_sb, cps, rl)
                nc.sync.dma_start(out_ext[b : b + 1, :], o_sb)

            pending = None
            for b in range(bl):
                lparts = smpool.tile([P, nchunk], f32, tag="lparts")
                cps = cps_pool.tile([1, DH], f32, tag="c")
                for i in range(nchunk):
                    if (b, i) in preload:
                        hbf, ht = preload.pop((b, i))
                    else:
                        hbf, ht = emit_load(b, i)
                    if stage >= 2:
                        pt = emit_scores(b, i, ht, lparts)
                    if stage >= 4:
                        if pending is not None:
                            emit_mm3(*pending)
                            if pending[1] == nchunk - 1:
                                emit_finalize(pending[0], *pending_fin)
                        pending = (b, i, hbf, pt, cps)
                pending_fin = (lparts, cps)
            if stage >= 4:
                emit_mm3(*pending)
                emit_finalize(pending[0], *pending_fin)
            else:
                for b in range(bl):
                    o_sb = outpool.tile([1, DH], f32, tag="osb")
                    nc.any.memset(o_sb, 0.0)
                    nc.sync.dma_start(out_ext[b : b + 1, :], o_sb)

    _legalize_waits(nc)
    return nc


def _get_nc():
    if "nc" not in _CACHE:
        _CACHE["nc"] = build_bass()
    return _CACHE["nc"]


def kernel(s, h, W_a, U_a, v_a):
    from concourse.bass_utils import run_bass_kernel_spmd

    s = np.ascontiguousarray(np.asarray(s, dtype=np.float32))
    h = np.ascontiguousarray(np.asarray(h, dtype=np.float32))
    W_a = np.ascontiguousarray(np.asarray(W_a, dtype=np.float32))
    U_a = np.ascontiguousarray(np.asarray(U_a, dtype=np.float32))
    v_a = np.ascontiguousarray(np.asarray(v_a, dtype=np.float32))

    nc = _get_nc()
    in_maps = []
    for c in range(NCORES):
        sl = slice(c * BL, (c + 1) * BL)
        in_maps.append(
            {"s": s[sl], "h": h[sl], "W_a": W_a, "U_a": U_a, "v_a": v_a}
        )
    res = run_bass_kernel_spmd(nc, in_maps, core_ids=list(range(NCORES)))
    outs = [res.results[c]["out"] for c in range(NCORES)]
    return np.concatenate(outs, axis=0).astype(np.float32)
